# revision 34
# baseline (speedup 1.0000x reference)
"""Trainium2 Bass kernel for nn_InternalMAFE_59270548684863.

Structure (v2):
  - Only branch 1 (p=7, n=288) of the reference affects the output; the
    n2=1008 branch feeds a dead projection and is never computed.
  - Batch-sharded over 8 cores (512 rows each); softmax normalizes over
    the batch axis, so per-(step, feature) exp-sums are AllReduced
    ([128,24] f32). Constant-shift softmax exp(s*scale - 50) avoids a
    cross-core max pass.
  - All de-interleave / transpose layout work is done host-side at
    sharding time: x and proj_len_w are fed to the device already
    transposed ([feature, batch] / [feature, seq]) in a packed 16-chunk
    layout where the seven 32-row tails (j in [256,288)) are packed
    4+3 into two 128-partition slots at offsets 32k. proj_len_b is fed
    pre-transposed so the projection bias is per-partition.
  - Device: bf16 cast-DMA loads -> logits via fused W_hk = h1@wk^T ->
    exp(+accum) -> AllReduce (overlapped with the v-matmuls) -> gated
    scan fused with the softmax normalization (scalar_tensor_tensor:
    y = t*recip + tanh(..)*sigmoid(..)) -> projection computed
    transposed (out^T[s,b], K packed to 16 chunks, bias added during
    PSUM evacuation) -> host transposes the output back.
  - The 32-row tail chunks run the elementwise pipeline (exp, multiply,
    scan) in slim offset-0 tiles (the scan chains consecutive steps, so
    they must share partitions); after each step's scan the tail is
    repacked into the 128-row projection tiles by SBUF->SBUF DMA.
"""

import math

import numpy as np

import concourse.bacc as bacc
import concourse.masks as masks
import concourse.mybir as mybir
import concourse.tile as tile
from concourse.bass_utils import run_bass_kernel_spmd

N_CORES = 8
B = 4096
BL = B // N_CORES  # 512 rows per core
INP = 2016
P1 = 7
N1 = 288
SEQ = 1024
SCALE = 1.0 / math.sqrt(N1)
SHIFT = -50.0
F32 = mybir.dt.float32
BF16 = mybir.dt.bfloat16
AF = mybir.ActivationFunctionType
ALU = mybir.AluOpType

NSLOT = 16
CH = [(0, 128), (128, 128), (256, 32)]


def xchunk(i, lt):
    """K-side: (slot, partition offset, count) of x chunk lt of step i."""
    if lt < 2:
        return 2 + 2 * i + lt, 0, 128
    if i < 4:
        return 0, 32 * i, 32
    return 1, 32 * (i - 4), 32


def pack_pos(i):
    """(pack tile index, partition offset) of step i's 32-row tail."""
    if i < 4:
        return 0, 32 * i
    return 1, 32 * (i - 4)


def build():
    nc = bacc.Bacc(
        "TRN2", target_bir_lowering=False, debug=False, num_devices=N_CORES
    )
    xp = nc.dram_tensor("xp", [128, NSLOT * BL], F32, kind="ExternalInput").ap()
    rkp = nc.dram_tensor("rkp", [128, NSLOT * SEQ], F32, kind="ExternalInput").ap()
    wp = nc.dram_tensor("wp", [128, 9 * N1], F32, kind="ExternalInput").ap()
    plbT = nc.dram_tensor("plbT", [128, 8], F32, kind="ExternalInput").ap()
    scal = nc.dram_tensor("scal", [1, 4], F32, kind="ExternalInput").ap()
    outT = nc.dram_tensor("outT", [SEQ, BL], F32, kind="ExternalOutput").ap()

    with tile.TileContext(nc) as tc:
        with (
            tc.tile_pool(name="const", bufs=1) as cpool,
            tc.tile_pool(name="data", bufs=1) as dpool,
            tc.tile_pool(name="stage", bufs=2) as spool,
            tc.tile_pool(name="dram", bufs=1, space="DRAM") as drpool,
        ):
            # ---------- constants / small loads ----------
            ident = cpool.tile([128, 128], BF16, tag="ident", name="ident")
            masks.make_identity(nc, ident[:])

            scal_sb = cpool.tile([1, 4], F32, tag="scal", name="scal")
            nc.sync.dma_start(scal_sb[:], scal[:])
            plb_sb = cpool.tile([128, 8], F32, tag="plb", name="plb")
            nc.sync.dma_start(plb_sb[:], plbT[:])

            shiftc = cpool.tile([128, 1], F32, tag="shiftc", name="shiftc")
            nc.vector.memset(shiftc[:], SHIFT)
            densb = cpool.tile([128, 24], F32, tag="densb", name="densb")
            nc.vector.memset(densb[:], 0.0)
            den_all = cpool.tile([128, 24], F32, tag="den_all", name="den_all")
            recip = cpool.tile([128, 24], F32, tag="recip", name="recip")
            ccA_in = drpool.tile([128, 12], F32)
            ccA_out = drpool.tile([128, 12], F32, addr_space="Shared")
            ccB_in = drpool.tile([128, 12], F32)
            ccB_out = drpool.tile([128, 12], F32, addr_space="Shared")

            # ---------- big inputs: fp32 HWDGE DMA + on-chip bf16 cast ----
            # (the SWDGE cast-DMA path measures ~3x below line rate)
            wpb = dpool.tile([128, 9 * N1], BF16, tag="wpb", name="wpb")
            xpb = dpool.tile([128, NSLOT * BL], BF16, tag="xpb", name="xpb")
            rkb = dpool.tile([128, NSLOT * SEQ], BF16, tag="rkb", name="rkb")
            wpf = spool.tile([128, 9 * N1], F32, tag="wpf", name="wpf")
            nc.sync.dma_start(wpf[:], wp[:])
            nc.vector.tensor_copy(wpb[:], wpf[:])
            for q in range(4):
                cols = slice(q * 4 * BL, (q + 1) * 4 * BL)
                xf = spool.tile([128, 4 * BL], F32, tag="xf", name=f"xf{q}")
                nc.sync.dma_start(xf[:], xp[:, cols])
                nc.vector.tensor_copy(xpb[:, cols], xf[:])
            # rk staging DMAs issue now (behind x on the sync ring); the
            # bf16 casts are emitted after phase A1 so exps keep priority.
            # 8 independent buffers: no cast->DMA coupling.
            rf_tiles = []
            for q in range(8):
                rf = spool.tile([128, 2 * SEQ], F32, tag=f"rf{q}",
                                name=f"rf{q}", bufs=1)
                nc.sync.dma_start(rf[:], rkp[:, q * 2 * SEQ : (q + 1) * 2 * SEQ])
                rf_tiles.append(rf)

            # ---------- persistent data tiles ----------
            # paired c0|c1 tiles per step: cols [0,512) = c0, [512,1024) = c1.
            # Each tile holds, in sequence (all in place): E = exp(logits),
            # then t = vT*E, then the scanned y.
            ysbP = [
                dpool.tile([128, 2 * BL], BF16, tag=f"ysP{i}", name=f"ysP{i}")
                for i in range(P1)
            ]
            # slim 32-row tail tiles per step (offset 0, scan-chainable)
            ysb2 = [
                dpool.tile([32, BL], BF16, tag=f"ys2_{i}", name=f"ys2_{i}")
                for i in range(P1)
            ]
            # projection-side packs of the tails (4+3 steps)
            ypack = [
                dpool.tile([128, BL], BF16, tag=f"yp{k}", name=f"yp{k}")
                for k in range(2)
            ]

            # ---------- phase W: gate bcast + W_hk^T prep ----------
            whkT = []  # [l-chunk] -> bf16 weights, lt=2 replicated 4x
            with tc.tile_pool(name="psW", bufs=2, space="PSUM") as psW:
                onesf = cpool.tile([1, 128], F32, tag="onesf", name="onesf")
                nc.vector.memset(onesf[:], 1.0)
                pbc = psW.tile([128, 512], F32, tag="psw", name="ps_bc")
                nc.tensor.matmul(
                    pbc[:, 0:4], onesf[:], scal_sb[:], start=True, stop=True
                )
                bcast = cpool.tile([128, 4], F32, tag="bcast", name="bcast")
                nc.vector.tensor_copy(bcast[:], pbc[:, 0:4])

                # h1T[l, j], wkT[l, m] via PE transposes of the packed weights
                h1T, wkT = [], []
                for lt, (l0, lc) in enumerate(CH):
                    ps = psW.tile([128, 512], BF16, tag="psw", name="ps_t")
                    for jt, (j0, jc) in enumerate(CH):
                        nc.tensor.transpose(
                            ps[0:lc, j0 : j0 + jc],
                            wpb[0:jc, jt * N1 + l0 : jt * N1 + l0 + lc],
                            ident[0:jc, 0:jc],
                        )
                    hT = cpool.tile([lc, N1], BF16, tag=f"h1T{lt}", name=f"h1T{lt}")
                    nc.vector.tensor_copy(hT[:], ps[0:lc, 0:N1])
                    h1T.append(hT)
                    ps2 = psW.tile([128, 512], BF16, tag="psw", name="ps_t2")
                    for mt, (m0, mc) in enumerate(CH):
                        nc.tensor.transpose(
                            ps2[0:lc, m0 : m0 + mc],
                            wpb[0:mc, (3 + mt) * N1 + l0 : (3 + mt) * N1 + l0 + lc],
                            ident[0:mc, 0:mc],
                        )
                    wTl = cpool.tile([lc, N1], BF16, tag=f"wkT{lt}", name=f"wkT{lt}")
                    nc.vector.tensor_copy(wTl[:], ps2[0:lc, 0:N1])
                    wkT.append(wTl)

                # W_hkT[m, j] = sum_l wk[m,l] h1[j,l]
                for mt, (m0, mc) in enumerate(CH):
                    pw = psW.tile([128, 512], F32, tag="psw", name="ps_whk")
                    for lt in range(3):
                        nc.tensor.matmul(
                            pw[0:mc, 0:N1],
                            wkT[lt][:, m0 : m0 + mc],
                            h1T[lt][:],
                            start=(lt == 0),
                            stop=(lt == 2),
                        )
                    wt = cpool.tile(
                        [128, N1], BF16, tag=f"whkT{mt}", name=f"whkT{mt}"
                    )
                    if mc == 128:
                        nc.vector.tensor_copy(wt[0:128, :], pw[0:128, 0:N1])
                    else:
                        for k in range(4):
                            nc.vector.tensor_copy(
                                wt[32 * k : 32 * k + 32, :], pw[0:32, 0:N1]
                            )
                    whkT.append(wt)

            # ---------- phase A1: logits + exp(+accum) ----------
            # all 8 PSUM banks are claimed here so the vT matmuls of phase A2
            # cannot be scheduled ahead of the logits stream
            with (
                tc.tile_pool(name="psL", bufs=3, space="PSUM") as psL,
                tc.tile_pool(name="psLc", bufs=2, space="PSUM") as psLc,
            ):
                for i in range(P1):
                    pst = psL.tile([128, 2 * BL], F32, tag="psl", name=f"pst{i}")
                    pstc = psLc.tile([32, BL], F32, tag="pslc", name=f"pstc{i}")
                    for jt, (j0, jc) in enumerate(CH):
                        if jt < 2:
                            dst = pst[0:128, jt * BL : (jt + 1) * BL]
                            eout = ysbP[i][0:128, jt * BL : (jt + 1) * BL]
                        else:
                            dst = pstc[0:32, :]
                            eout = ysb2[i][0:32, :]
                        for lt in range(3):
                            slot, off, cnt = xchunk(i, lt)
                            nc.tensor.matmul(
                                dst,
                                whkT[lt][off : off + cnt, j0 : j0 + jc],
                                xpb[off : off + cnt, slot * BL : (slot + 1) * BL],
                                start=(lt == 0),
                                stop=(lt == 2),
                                tile_position=(off, 0) if off == 96 else None,
                            )
                        col = i * 3 + jt
                        nc.scalar.activation(
                            eout,
                            dst,
                            AF.Exp,
                            bias=shiftc[0:jc, 0:1],
                            scale=SCALE,
                            accum_out=densb[0:jc, col : col + 1],
                        )
                    if i == 3:
                        # first-half AllReduce (steps 0-3) fires mid-phase;
                        # bounce DMAs ride the ACT HWDGE ring (the sync ring
                        # is busy streaming x/rk)
                        nc.scalar.dma_start(ccA_in[:], densb[:, 0:12])
                        nc.gpsimd.collective_compute(
                            "AllReduce",
                            ALU.add,
                            replica_groups=[list(range(N_CORES))],
                            ins=[ccA_in[:]],
                            outs=[ccA_out[:]],
                        )

            # second-half AllReduce (steps 4-6)
            nc.scalar.dma_start(ccB_in[:], densb[:, 12:24])
            nc.gpsimd.collective_compute(
                "AllReduce",
                ALU.add,
                replica_groups=[list(range(N_CORES))],
                ins=[ccB_in[:]],
                outs=[ccB_out[:]],
            )
            nc.scalar.dma_start(den_all[:, 0:12], ccA_out[:])
            nc.vector.reciprocal(recip[:, 0:12], den_all[:, 0:12])
            nc.scalar.dma_start(den_all[:, 12:24], ccB_out[:])
            nc.vector.reciprocal(recip[:, 12:24], den_all[:, 12:24])

            # rk bf16 casts (emitted after A1 so the exps keep ACT priority;
            # all on the scalar engine, which idles through the vT phase)
            for q in range(8):
                cols = slice(q * 2 * SEQ, (q + 1) * 2 * SEQ)
                nc.scalar.copy(rkb[:, cols], rf_tiles[q][:])

            # ---------- phase A2: vT = (x_i @ wv)^T ; raw t = vT * E ----------
            with (
                tc.tile_pool(name="psV", bufs=2, space="PSUM") as psV,
                tc.tile_pool(name="psVc", bufs=2, space="PSUM") as psVc,
            ):
                for i in range(P1):
                    pv = psV.tile([128, 2 * BL], F32, tag="psv", name=f"pv{i}")
                    pvc = psVc.tile([32, BL], F32, tag="psvc", name=f"pvc{i}")
                    for nt, (n0, ncnt) in enumerate(CH):
                        if nt < 2:
                            dst = pv[0:128, nt * BL : (nt + 1) * BL]
                        else:
                            dst = pvc[0:32, :]
                        for lt in range(3):
                            slot, off, cnt = xchunk(i, lt)
                            nc.tensor.matmul(
                                dst,
                                wpb[
                                    off : off + cnt,
                                    (6 + lt) * N1 + n0 : (6 + lt) * N1 + n0 + ncnt,
                                ],
                                xpb[off : off + cnt, slot * BL : (slot + 1) * BL],
                                start=(lt == 0),
                                stop=(lt == 2),
                                tile_position=(off, 0) if off == 96 else None,
                            )
                    # paired multiply for c0|c1, slim for the tail (in place:
                    # the tiles hold E before, raw t after)
                    nc.vector.tensor_mul(ysbP[i][:], pv[0:128, :], ysbP[i][:])
                    nc.vector.tensor_mul(
                        ysb2[i][0:32, :], pvc[0:32, :], ysb2[i][0:32, :]
                    )

            # ---------- phase B: gated scan (fused normalize) + projection ----
            with (
                tc.tile_pool(name="gat", bufs=2) as gpool,
                tc.tile_pool(name="osb", bufs=3) as opool,
                tc.tile_pool(name="psP", bufs=1, space="PSUM") as psP,
            ):
                pps = [
                    psP.tile([128, BL], F32, tag=f"pp{st}", name=f"pp{st}")
                    for st in range(8)
                ]
                proj_started = [False] * 8

                proj_sources = []

                def proj(src_tile, coff, cnt, rk_slot, last):
                    # s-tiles 0-6 accumulate during the scan; s-tile 7 runs
                    # post-scan (bank 7 hosts the warm-keeper dummies)
                    proj_sources.append((src_tile, coff, cnt, rk_slot))
                    for st in range(7):
                        c0 = rk_slot * SEQ + st * 128
                        nc.tensor.matmul(
                            pps[st][:],
                            rkb[0:cnt, c0 : c0 + 128],
                            src_tile[0:cnt, coff : coff + BL],
                            start=not proj_started[st],
                            stop=last,
                        )
                        proj_started[st] = True

                def scan_pair(i):
                    """paired c0|c1 scan update on ysbP[i] (in place)."""
                    if i == 0:
                        for c in range(2):
                            h = slice(c * BL, (c + 1) * BL)
                            nc.vector.tensor_scalar_mul(
                                ysbP[0][:, h], ysbP[0][:, h],
                                recip[0:128, c : c + 1],
                            )
                    else:
                        tt = gpool.tile([128, 2 * BL], BF16, tag="tt", name="tt")
                        ts = gpool.tile([128, 2 * BL], BF16, tag="ts", name="ts")
                        nc.scalar.activation(
                            tt[:], ysbP[i - 1][:], AF.Tanh,
                            bias=bcast[0:128, 2:3], scale=bcast[0:128, 0:1],
                        )
                        nc.scalar.activation(
                            ts[:], ysbP[i - 1][:], AF.Sigmoid,
                            bias=bcast[0:128, 3:4], scale=bcast[0:128, 1:2],
                        )
                        nc.vector.tensor_mul(tt[:], tt[:], ts[:])
                        for c in range(2):
                            h = slice(c * BL, (c + 1) * BL)
                            col = i * 3 + c
                            nc.vector.tensor_scalar_mul(
                                ysbP[i][:, h], ysbP[i][:, h],
                                recip[0:128, col : col + 1],
                            )
                        nc.vector.tensor_add(ysbP[i][:], ysbP[i][:], tt[:])

                def scan_slim(i):
                    seg = slice(0, 32)
                    col = i * 3 + 2
                    if i == 0:
                        nc.vector.tensor_scalar_mul(
                            ysb2[0][seg, :], ysb2[0][seg, :],
                            recip[seg, col : col + 1],
                        )
                    else:
                        tt = gpool.tile([32, BL], BF16, tag="tt2", name="tt2")
                        ts = gpool.tile([32, BL], BF16, tag="ts2", name="ts2")
                        nc.scalar.activation(
                            tt[seg, :], ysb2[i - 1][seg, :], AF.Tanh,
                            bias=bcast[seg, 2:3], scale=bcast[seg, 0:1],
                        )
                        nc.scalar.activation(
                            ts[seg, :], ysb2[i - 1][seg, :], AF.Sigmoid,
                            bias=bcast[seg, 3:4], scale=bcast[seg, 1:2],
                        )
                        nc.vector.tensor_mul(tt[seg, :], tt[seg, :], ts[seg, :])
                        nc.vector.scalar_tensor_tensor(
                            ysb2[i][seg, :], ysb2[i][seg, :],
                            recip[seg, col : col + 1], tt[seg, :],
                            ALU.mult, ALU.add,
                        )

                for i in range(P1):
                    scan_pair(i)
                    scan_slim(i)
                    for c in range(2):
                        proj(ysbP[i], c * BL, 128, 2 + 2 * i + c, False)
                    # repack the finished tail for the projection
                    pk, off = pack_pos(i)
                    nc.sync.dma_start(
                        ypack[pk][off : off + 32, :], ysb2[i][0:32, :]
                    )
                    if i == 3:
                        proj(ypack[0], 0, 128, 0, False)
                    elif i == 6:
                        proj(ypack[1], 0, 96, 1, True)

                # warm-keeper (emitted after the scan so its priority is
                # BELOW the real scan/proj work): a self-paced DVE copy chain
                # feeds the PE a tiny matmul every ~0.6us across the
                # vT->scan AllReduce gap so HAM does not re-throttle the PE
                # clock before the projection burst. Targets bank 7, whose
                # real projection runs post-scan.
                scr = gpool.tile([128, 2048], BF16, tag="scr", name="scr",
                                 bufs=1)
                nc.vector.tensor_copy(scr[:, 0 : 2 * BL], ysbP[P1 - 1][:])
                for k in range(20):
                    nc.vector.tensor_copy(scr[:], scr[:])
                    nc.tensor.matmul(
                        pps[7][0:64, 0:64],
                        ident[0:128, 0:64],
                        scr[0:128, (k % 16) * 128 : (k % 16) * 128 + 64],
                        start=True,
                        stop=True,
                        skip_group_check=True,
                    )

                # s-tile 7 projection (after the dummies release bank 7)
                for n, (src_tile, coff, cnt, rk_slot) in enumerate(proj_sources):
                    c0 = rk_slot * SEQ + 7 * 128
                    nc.tensor.matmul(
                        pps[7][:],
                        rkb[0:cnt, c0 : c0 + 128],
                        src_tile[0:cnt, coff : coff + BL],
                        start=(n == 0),
                        stop=(n == len(proj_sources) - 1),
                    )

                # evacuate + bias + store (transposed out); split ACT/DVE
                for st in range(8):
                    ob = opool.tile([128, BL], F32, tag="ob", name=f"ob{st}")
                    if st % 2 == 0:
                        nc.scalar.activation(
                            ob[:],
                            pps[st][:],
                            AF.Identity,
                            bias=plb_sb[:, st : st + 1],
                        )
                    else:
                        nc.vector.tensor_scalar_add(
                            ob[:], pps[st][:], plb_sb[:, st : st + 1]
                        )
                    nc.sync.dma_start(outT[st * 128 : (st + 1) * 128, :], ob[:])

    nc.compile()
    return nc


def _pack_feat(src, ncols):
    """[7, 288, ncols] (i, j, cols) -> packed [128, 16*ncols]."""
    out = np.zeros((128, NSLOT, ncols), dtype=np.float32)
    for i in range(P1):
        out[:, 2 + 2 * i, :] = src[i, 0:128, :]
        out[:, 2 + 2 * i + 1, :] = src[i, 128:256, :]
    for k in range(4):
        out[32 * k : 32 * k + 32, 0, :] = src[k, 256:288, :]
    for k in range(3):
        out[32 * k : 32 * k + 32, 1, :] = src[4 + k, 256:288, :]
    return np.ascontiguousarray(out.reshape(128, NSLOT * ncols))


def _prep_host(inputs):
    f32 = np.float32
    x = np.ascontiguousarray(inputs["x"], dtype=f32)
    plw = np.ascontiguousarray(inputs["proj_len_w"], dtype=f32)
    plb = np.ascontiguousarray(inputs["proj_len_b"], dtype=f32)
    h1 = np.ascontiguousarray(inputs["h1"], dtype=f32)
    wk = np.ascontiguousarray(inputs["w_k1"], dtype=f32)
    wv = np.ascontiguousarray(inputs["w_v1"], dtype=f32)

    # proj_len_w de-interleaved + transposed: [7, 288, 1024]
    plwT = plw.reshape(SEQ, N1, P1).transpose(2, 1, 0)
    rkp = _pack_feat(plwT, SEQ)

    wp = np.zeros((128, 9, N1), dtype=f32)
    for base, W in ((0, h1), (3, wk), (6, wv)):
        wp[:, base + 0, :] = W[0:128, :]
        wp[:, base + 1, :] = W[128:256, :]
        for k in range(4):
            wp[32 * k : 32 * k + 32, base + 2, :] = W[256:288, :]
    wp = np.ascontiguousarray(wp.reshape(128, 9 * N1))

    plbT = np.ascontiguousarray(plb.reshape(8, 128).T)
    scal = np.array(
        [[inputs["alpha1"][0], inputs["alpha2"][0],
          inputs["beta1"][0], inputs["beta2"][0]]],
        dtype=f32,
    )

    rep = {"rkp": rkp, "wp": wp, "plbT": plbT, "scal": scal}
    in_maps = []
    for c in range(N_CORES):
        xc = x[c * BL : (c + 1) * BL]  # [512, 2016]
        xT = xc.reshape(BL, N1, P1).transpose(2, 1, 0)  # [7, 288, 512]
        in_maps.append({"xp": _pack_feat(xT, BL), **rep})
    return in_maps


_NC = None


def _get_nc():
    global _NC
    if _NC is None:
        _NC = build()
    return _NC


def run(inputs, trace=False):
    nc = _get_nc()
    in_maps = _prep_host(inputs)
    res = run_bass_kernel_spmd(
        nc, in_maps, core_ids=list(range(N_CORES)), trace=trace
    )
    full = np.concatenate(
        [res.results[c]["outT"].T for c in range(N_CORES)], axis=0
    )
    return np.ascontiguousarray(full), res


def kernel(**inputs):
    full, _ = run(inputs, trace=False)
    return full
